# revision 9
# baseline (speedup 1.0000x reference)
"""Trainium2 Bass kernel for Local_Scale_Attention.

Problem (hardcoded shapes):
  x:          (2048, 84, 256) f32
  W_qkv:      (256, 768) f32,  b_qkv: (768,) f32
  W_proj:     (256, 256) f32,  b_proj: (256,) f32
  bias_table: (207, 8) f32,    rel_index: (84, 84) i32
  out:        (2048, 84, 256) f32

  qkv = x @ W_qkv + b_qkv -> (B,84,3,8,32); attn = softmax(q*scale @ k^T + bias)
  out = (attn @ v) @ W_proj + b_proj

Sharding: data-parallel over batch across 8 cores (256 batches/core).

Per-core device pipeline (feature-major, S^T orientation):
  - host passes xT (256, 21504) feature-major (transpose done on host)
  - qkv^T = W_qkv^T @ xT per group of 6 batches (fp32r matmuls, N=504)
    q,k evacuated feature-major bf16 (heads 32-aligned on partitions);
    v computed token-major per batch (M=84) and evacuated bf16
  - S^T_{b,h} = k_h^T.T @ q_h^T  (bf16, K=32, row-packed 4 heads via
    tile_position; out [84 k-tok, 84 q-tok] in 4-bank PSUM fill of 3 batches)
  - E0 = exp(scale * S^T) on ACT (one op per fill), E = E0 * exp(bias^T)
    on DVE (4x bf16)
  - O_un^T = V^T-block matmuls col-packed 4 heads (lhsT=V [84,32] bf16),
    plus denominator rows via lhsT=ones trick replicated over 32 partitions
  - O^T = O_un / D (one DVE divide per 128-row block), feature-major
  - y^T = W_proj^T @ O^T (fp32r, N=504) -> DMA out yT (256, 21504)
  - b_qkv(q,k) folded into evac; b_v and b_proj folded into host-side
    bias add (softmax weights sum to 1 so v-bias passes through attn).
"""

import math

import numpy as np
import ml_dtypes

import concourse.bass as bass
import concourse.bacc as bacc
import concourse.mybir as mybir
import concourse.tile as tile
from concourse.bass_utils import run_bass_kernel_spmd

F32 = mybir.dt.float32
F32R = mybir.dt.float32r
BF16 = mybir.dt.bfloat16

N_CORES = 8
B_TOTAL = 2048
B_SHARD = B_TOTAL // N_CORES  # 256
N_TOK = 84
DIM = 256
H = 8
HD = 32
SCALE = float(N_TOK) ** -0.5

GROUP = 6           # batches per outer group (N = 6*84 = 504 <= 512)
FILL = 3            # batches per S-psum fill (24 S-matrices in 4 banks)


def _r(ap):
    """bitcast an fp32 AP to fp32r for fast matmul"""
    return ap.bitcast(F32R)


def build_nc(n_batches=B_SHARD):
    T_ALL = n_batches * N_TOK
    nc = bacc.Bacc("TRN2", target_bir_lowering=False, debug=False)

    xT = nc.declare_dram_parameter("xT", [DIM, T_ALL], BF16, isOutput=False)
    wqkv = nc.declare_dram_parameter("wqkv", [DIM, 3 * DIM], BF16, isOutput=False)
    bqkv = nc.declare_dram_parameter("bqkv", [3 * DIM], F32, isOutput=False)
    wproj = nc.declare_dram_parameter("wproj", [DIM, DIM], BF16, isOutput=False)
    # exp(bias^T) pre-aligned to the S-fill layout: [84(k), 4(bank), 504(6 slots x 84)]
    ebias = nc.declare_dram_parameter("ebias", [N_TOK, 4, 504], BF16, isOutput=False)
    yT = nc.declare_dram_parameter("yT", [DIM, T_ALL], F32, isOutput=True)

    groups = []
    b0 = 0
    while b0 < n_batches:
        groups.append((b0, min(GROUP, n_batches - b0)))
        b0 += GROUP

    with tile.TileContext(nc) as tc:
        with (
            tc.tile_pool(name="const", bufs=1) as const,
            tc.tile_pool(name="xin", bufs=3) as xin,
            tc.tile_pool(name="qk", bufs=2) as qk_pool,
            tc.tile_pool(name="vsb", bufs=2) as v_pool,
            tc.tile_pool(name="esb", bufs=2) as e_pool,
            tc.tile_pool(name="otsb", bufs=2) as ot_pool,
            tc.tile_pool(name="ysb", bufs=2) as y_pool,
            tc.tile_pool(name="mm_ps", bufs=2, space="PSUM") as mm_ps,
            tc.tile_pool(name="s_ps", bufs=1, space="PSUM") as s_ps_pool,
            tc.tile_pool(name="av_ps", bufs=1, space="PSUM") as av_ps_pool,
        ):
            # ---- static tiles ----
            wqkv_sb = const.tile([128, 2, 3 * DIM], BF16)
            nc.sync.dma_start(wqkv_sb, wqkv.rearrange("(kc p) m -> p kc m", p=128))
            wproj_sb = const.tile([128, 2, DIM], BF16)
            nc.sync.dma_start(wproj_sb, wproj.rearrange("(kc p) m -> p kc m", p=128))
            bqkv_sb = const.tile([128, 6], F32)
            nc.sync.dma_start(bqkv_sb, bqkv.rearrange("(m p) -> p m", p=128))
            eb_sb = const.tile([N_TOK, 4, 504], BF16)
            nc.sync.dma_start(eb_sb, ebias[:])
            ones_sb = const.tile([N_TOK, HD], BF16)
            nc.vector.memset(ones_sb, 1.0)

            for (g0, g) in groups:
                TG = g * N_TOK
                T0 = g0 * N_TOK

                # ---- load x^T slice ----
                xt = xin.tile([128, 2, GROUP * N_TOK], BF16)
                nc.sync.dma_start(
                    xt[:, :, :TG],
                    xT[:, T0:T0 + TG].rearrange("(kc p) t -> p kc t", p=128),
                )

                # ---- q,k feature-major (chunks of 128 f_out) ----
                qk_sb = qk_pool.tile([128, 4, GROUP * N_TOK], BF16)
                for m in range(4):  # 0,1 = q; 2,3 = k
                    ps = mm_ps.tile([128, 512], F32, tag="mmps")
                    for kc in range(2):
                        nc.tensor.matmul(
                            ps[:, :TG],
                            wqkv_sb[:, kc, m * 128:(m + 1) * 128],
                            xt[:, kc, :TG],
                            start=(kc == 0), stop=(kc == 1),
                        )
                    nc.any.tensor_scalar_add(
                        qk_sb[:, m, :TG], ps[:, :TG], bqkv_sb[:, m:m + 1]
                    )

                # ---- v token-major, per pair of batches ----
                v_sb = v_pool.tile([N_TOK, GROUP, DIM], BF16)
                for p2 in range((g + 1) // 2):
                    nb2 = min(2, g - 2 * p2)
                    psv = mm_ps.tile([128, 512], F32, tag="mmps")
                    for jj in range(nb2):
                        j = 2 * p2 + jj
                        for kc in range(2):
                            nc.tensor.matmul(
                                psv[:N_TOK, jj * DIM:(jj + 1) * DIM],
                                xt[:, kc, j * N_TOK:(j + 1) * N_TOK],
                                wqkv_sb[:, kc, 2 * DIM:3 * DIM],
                                start=(kc == 0), stop=(kc == 1),
                            )
                    nc.any.tensor_copy(
                        v_sb[:, 2 * p2:2 * p2 + nb2, :],
                        psv[:N_TOK, :nb2 * DIM].rearrange(
                            "p (j c) -> p j c", c=DIM
                        ),
                    )

                # ---- attention per fill of up to 3 batches ----
                e_tiles = []   # (e_tile, fill_start_batch_local)
                for f0 in range(0, g, FILL):
                    nb = min(FILL, g - f0)
                    s_ps = s_ps_pool.tile([N_TOK, 2048], F32)
                    for jl in range(nb):
                        j = f0 + jl
                        for h in range(H):
                            bank = h % 4
                            slot = 2 * jl + h // 4
                            hq = h // 4
                            hp = 32 * (h % 4)
                            nc.tensor.matmul(
                                s_ps[:, 512 * bank + 84 * slot:
                                     512 * bank + 84 * slot + 84],
                                qk_sb[hp:hp + 32, 2 + hq, j * N_TOK:(j + 1) * N_TOK],
                                qk_sb[hp:hp + 32, 0 + hq, j * N_TOK:(j + 1) * N_TOK],
                                start=True, stop=True,
                                tile_position=(hp, 0),
                            )
                    vcols = 84 * 2 * nb
                    s_view = s_ps.rearrange("p (bk c) -> p bk c", c=512)
                    e0 = e_pool.tile([N_TOK, 4, 504], BF16, tag="e0")
                    nc.scalar.activation(
                        e0[:, :, :vcols], s_view[:, :, :vcols],
                        mybir.ActivationFunctionType.Exp, scale=SCALE,
                    )
                    e = e_pool.tile([N_TOK, 4, 504], BF16, tag="e")
                    nc.vector.tensor_tensor(
                        e[:, :, :vcols], e0[:, :, :vcols], eb_sb[:, :, :vcols],
                        mybir.AluOpType.mult,
                    )
                    e_tiles.append((e, f0))

                # ---- AV + denominators, col-packed 4 heads per 128-row block ----
                ot_tiles = []
                for hg in range(2):
                    av = av_ps_pool.tile([128, 1024], F32)
                    for j in range(g):
                        e, f0 = e_tiles[j // FILL]
                        jl = j - f0
                        for hh in range(4):
                            h = 4 * hg + hh
                            bank = h % 4
                            slot = 2 * jl + h // 4
                            esl = e[:, bank, 84 * slot:84 * slot + 84]
                            # O_un^T block
                            nc.tensor.matmul(
                                av[32 * hh:32 * hh + 32, 84 * j:84 * j + 84],
                                v_sb[:, j, 32 * h:32 * h + 32],
                                esl, start=True, stop=True,
                                tile_position=(0, 32 * hh),
                            )
                            # denominator replicated over the 32 partitions
                            nc.tensor.matmul(
                                av[32 * hh:32 * hh + 32,
                                   512 + 84 * j:512 + 84 * j + 84],
                                ones_sb, esl, start=True, stop=True,
                                tile_position=(0, 32 * hh),
                            )
                    r_sb = ot_pool.tile([128, GROUP * N_TOK], F32, tag=f"d{hg}")
                    nc.vector.reciprocal(r_sb[:, :TG], av[:, 512:512 + TG])
                    ot = ot_pool.tile([128, GROUP * N_TOK], BF16, tag=f"ot{hg}")
                    nc.vector.tensor_tensor(
                        ot[:, :TG], av[:, :TG], r_sb[:, :TG],
                        mybir.AluOpType.mult,
                    )
                    ot_tiles.append(ot)

                # ---- projection ----
                y_sb = y_pool.tile([128, 2, GROUP * N_TOK], F32)
                for m in range(2):
                    psy = mm_ps.tile([128, 512], F32, tag="mmps")
                    for kc in range(2):
                        nc.tensor.matmul(
                            psy[:, :TG],
                            wproj_sb[:, kc, m * 128:(m + 1) * 128],
                            ot_tiles[kc][:, :TG],
                            start=(kc == 0), stop=(kc == 1),
                        )
                    nc.any.tensor_copy(y_sb[:, m, :TG], psy[:, :TG])

                nc.sync.dma_start(
                    yT[:, T0:T0 + TG].rearrange("(m p) t -> p m t", p=128),
                    y_sb[:, :, :TG],
                )

    nc.compile()
    return nc


def _host_prep(x, W_qkv, b_qkv, W_proj, b_proj, bias_table, rel_index, n_batches):
    """Build the per-core input dict pieces shared across cores."""
    bias_full = bias_table[rel_index]          # (84, 84, 8): [q, k, h]
    bias_full = np.transpose(bias_full, (2, 0, 1))  # [h, q, k]
    eb = np.zeros((N_TOK, 4, 504), dtype=np.float32)
    for bank in range(4):
        for slot in range(6):
            h = bank + 4 * (slot % 2)
            # eb[k, bank, 84*slot + q] = exp(bias[h, q, k])
            eb[:, bank, 84 * slot:84 * slot + 84] = np.exp(bias_full[h].T)
    eb = eb.astype(ml_dtypes.bfloat16)

    shared = {
        "wqkv": np.ascontiguousarray(W_qkv).astype(ml_dtypes.bfloat16),
        "bqkv": np.ascontiguousarray(b_qkv, dtype=np.float32),
        "wproj": np.ascontiguousarray(W_proj).astype(ml_dtypes.bfloat16),
        "ebias": eb,
    }
    return shared


_NC_CACHE = {}


def kernel(x, W_qkv, b_qkv, W_proj, b_proj, bias_table, rel_index,
           n_batches_per_core=B_SHARD):
    x = np.asarray(x, dtype=np.float32)
    W_qkv = np.asarray(W_qkv, dtype=np.float32)
    b_qkv = np.asarray(b_qkv, dtype=np.float32)
    W_proj = np.asarray(W_proj, dtype=np.float32)
    b_proj = np.asarray(b_proj, dtype=np.float32)
    bias_table = np.asarray(bias_table, dtype=np.float32)
    rel_index = np.asarray(rel_index)

    nb = n_batches_per_core
    shared = _host_prep(x, W_qkv, b_qkv, W_proj, b_proj, bias_table,
                        rel_index, nb)

    if nb not in _NC_CACHE:
        _NC_CACHE[nb] = build_nc(nb)
    nc = _NC_CACHE[nb]

    in_maps = []
    for c in range(N_CORES):
        xs = x[c * nb:(c + 1) * nb].reshape(nb * N_TOK, DIM)
        xTc = np.ascontiguousarray(xs.T).astype(ml_dtypes.bfloat16)
        m = dict(shared)
        m["xT"] = xTc
        in_maps.append(m)

    res = run_bass_kernel_spmd(nc, in_maps, core_ids=list(range(N_CORES)))

    # combined output bias: b_proj plus v-bias passed through attention
    bias_out = (b_qkv[2 * DIM:3 * DIM] @ W_proj + b_proj).astype(np.float32)

    outs = []
    for c in range(N_CORES):
        yTc = res.results[c]["yT"]               # (256, nb*84)
        y = np.ascontiguousarray(yTc.T).reshape(nb, N_TOK, DIM)
        outs.append(y)
    out = np.concatenate(outs, axis=0)
    out += bias_out[None, None, :]
    return out.astype(np.float32)


if __name__ == "__main__":
    # smoke test with tiny batch count per core
    rng = np.random.default_rng(0)
    nb = 6
    B = N_CORES * nb
    x = rng.standard_normal((B, N_TOK, DIM), dtype=np.float32)
    W_qkv = rng.standard_normal((DIM, 3 * DIM), dtype=np.float32) * 0.02
    b_qkv = np.zeros(3 * DIM, np.float32)
    W_proj = rng.standard_normal((DIM, DIM), dtype=np.float32) * 0.02
    b_proj = np.zeros(DIM, np.float32)
    bias_table = rng.standard_normal((207, H), dtype=np.float32) * 0.02
    # rebuild rel_index like the reference
    SQ = [64, 16, 4]
    offset = [0]
    for i in range(2):
        offset.append(sum(SQ[-i - 1:]))
    off_h = np.concatenate([np.full(SQ[i], offset[i], dtype=np.int64) for i in range(3)])
    off_w = np.concatenate([np.full(SQ[i], offset[-i - 1], dtype=np.int64) for i in range(3)])
    ch = np.arange(N_TOK)
    cw = np.arange(N_TOK)[::-1]
    rel_index = (ch[:, None] + cw[None, :] + off_h[:, None] + off_w[None, :]).astype(np.int32)

    out = kernel(x, W_qkv, b_qkv, W_proj, b_proj, bias_table, rel_index,
                 n_batches_per_core=nb)
    print("out", out.shape, out.dtype)


# revision 11
# speedup vs baseline: 2.0332x; 2.0332x over previous
"""Trainium2 Bass kernel for Local_Scale_Attention.

Problem (hardcoded shapes):
  x:          (2048, 84, 256) f32
  W_qkv:      (256, 768) f32,  b_qkv: (768,) f32
  W_proj:     (256, 256) f32,  b_proj: (256,) f32
  bias_table: (207, 8) f32,    rel_index: (84, 84) i32
  out:        (2048, 84, 256) f32

  qkv = x @ W_qkv + b_qkv -> (B,84,3,8,32); attn = softmax(q*scale @ k^T + bias)
  out = (attn @ v) @ W_proj + b_proj

Sharding: data-parallel over batch across 8 cores (256 batches/core).

Per-core device pipeline (feature-major, S^T orientation):
  - host passes xT (256, 21504) feature-major (transpose done on host)
  - qkv^T = W_qkv^T @ xT per group of 6 batches (fp32r matmuls, N=504)
    q,k evacuated feature-major bf16 (heads 32-aligned on partitions);
    v computed token-major per batch (M=84) and evacuated bf16
  - S^T_{b,h} = k_h^T.T @ q_h^T  (bf16, K=32, row-packed 4 heads via
    tile_position; out [84 k-tok, 84 q-tok] in 4-bank PSUM fill of 3 batches)
  - E0 = exp(scale * S^T) on ACT (one op per fill), E = E0 * exp(bias^T)
    on DVE (4x bf16)
  - O_un^T = V^T-block matmuls col-packed 4 heads (lhsT=V [84,32] bf16),
    plus denominator rows via lhsT=ones trick replicated over 32 partitions
  - O^T = O_un / D (one DVE divide per 128-row block), feature-major
  - y^T = W_proj^T @ O^T (fp32r, N=504) -> DMA out yT (256, 21504)
  - b_qkv(q,k) folded into evac; b_v and b_proj folded into host-side
    bias add (softmax weights sum to 1 so v-bias passes through attn).
"""

import math

import numpy as np
import ml_dtypes

import concourse.bass as bass
import concourse.bacc as bacc
import concourse.mybir as mybir
import concourse.tile as tile
from concourse.bass_utils import run_bass_kernel_spmd

F32 = mybir.dt.float32
F32R = mybir.dt.float32r
BF16 = mybir.dt.bfloat16

N_CORES = 8
B_TOTAL = 2048
B_SHARD = B_TOTAL // N_CORES  # 256
N_TOK = 84
DIM = 256
H = 8
HD = 32
SCALE = float(N_TOK) ** -0.5

GROUP = 6           # batches per outer group (N = 6*84 = 504 <= 512)
FILL = 3            # batches per S-psum fill (24 S-matrices in 4 banks)


def _r(ap):
    """bitcast an fp32 AP to fp32r for fast matmul"""
    return ap.bitcast(F32R)


def build_nc(n_batches=B_SHARD):
    T_ALL = n_batches * N_TOK
    nc = bacc.Bacc("TRN2", target_bir_lowering=False, debug=False)

    xT = nc.declare_dram_parameter("xT", [DIM, T_ALL], BF16, isOutput=False)
    wqkv = nc.declare_dram_parameter("wqkv", [DIM, 3 * DIM], BF16, isOutput=False)
    bqkv = nc.declare_dram_parameter("bqkv", [3 * DIM], F32, isOutput=False)
    wproj = nc.declare_dram_parameter("wproj", [DIM, DIM], BF16, isOutput=False)
    # exp(bias^T) pre-aligned to the S-fill layout: [84(k), 4(bank), 504(6 slots x 84)]
    ebias = nc.declare_dram_parameter("ebias", [N_TOK, 4, 504], BF16, isOutput=False)
    yT = nc.declare_dram_parameter("yT", [DIM, T_ALL], F32, isOutput=True)

    groups = []
    b0 = 0
    while b0 < n_batches:
        groups.append((b0, min(GROUP, n_batches - b0)))
        b0 += GROUP

    with tile.TileContext(nc) as tc:
        with (
            tc.tile_pool(name="const", bufs=1) as const,
            tc.tile_pool(name="xin", bufs=3) as xin,
            tc.tile_pool(name="qk", bufs=2) as qk_pool,
            tc.tile_pool(name="vsb", bufs=2) as v_pool,
            tc.tile_pool(name="e0sb", bufs=2) as e0_pool,
            tc.tile_pool(name="esb", bufs=4) as e_pool,
            tc.tile_pool(name="otsb", bufs=2) as ot_pool,
            tc.tile_pool(name="ysb", bufs=2) as y_pool,
            tc.tile_pool(name="mm_ps", bufs=2, space="PSUM") as mm_ps,
            tc.tile_pool(name="s_ps", bufs=1, space="PSUM") as s_ps_pool,
            tc.tile_pool(name="av_ps", bufs=1, space="PSUM") as av_ps_pool,
        ):
            # ---- static tiles ----
            wqkv_sb = const.tile([128, 2, 3 * DIM], BF16)
            nc.sync.dma_start(wqkv_sb, wqkv.rearrange("(kc p) m -> p kc m", p=128))
            wproj_sb = const.tile([128, 2, DIM], BF16)
            nc.sync.dma_start(wproj_sb, wproj.rearrange("(kc p) m -> p kc m", p=128))
            bqkv_sb = const.tile([128, 6], F32)
            nc.sync.dma_start(bqkv_sb, bqkv.rearrange("(m p) -> p m", p=128))
            eb_sb = const.tile([N_TOK, 4, 504], BF16)
            nc.sync.dma_start(eb_sb, ebias[:])
            ones_sb = const.tile([N_TOK, HD], BF16)
            nc.vector.memset(ones_sb, 1.0)

            # ---------- pipelined stage helpers ----------
            def emit_qkv(g0, g):
                TG = g * N_TOK
                T0 = g0 * N_TOK
                xt = xin.tile([128, 2, GROUP * N_TOK], BF16)
                nc.sync.dma_start(
                    xt[:, :, :TG],
                    xT[:, T0:T0 + TG].rearrange("(kc p) t -> p kc t", p=128),
                )
                qk_sb = qk_pool.tile([128, 4, GROUP * N_TOK], BF16)
                for m in range(4):  # 0,1 = q; 2,3 = k
                    ps = mm_ps.tile([128, 512], F32, tag="mmps")
                    for kc in range(2):
                        nc.tensor.matmul(
                            ps[:, :TG],
                            wqkv_sb[:, kc, m * 128:(m + 1) * 128],
                            xt[:, kc, :TG],
                            start=(kc == 0), stop=(kc == 1),
                        )
                    nc.any.tensor_scalar_add(
                        qk_sb[:, m, :TG], ps[:, :TG], bqkv_sb[:, m:m + 1]
                    )
                return xt, qk_sb

            def emit_v(xt, g):
                v_sb = v_pool.tile([N_TOK, GROUP, DIM], BF16)
                for p2 in range((g + 1) // 2):
                    nb2 = min(2, g - 2 * p2)
                    psv = mm_ps.tile([128, 512], F32, tag="mmps")
                    for jj in range(nb2):
                        j = 2 * p2 + jj
                        for kc in range(2):
                            nc.tensor.matmul(
                                psv[:N_TOK, jj * DIM:(jj + 1) * DIM],
                                xt[:, kc, j * N_TOK:(j + 1) * N_TOK],
                                wqkv_sb[:, kc, 2 * DIM:3 * DIM],
                                start=(kc == 0), stop=(kc == 1),
                            )
                    nc.any.tensor_copy(
                        v_sb[:, 2 * p2:2 * p2 + nb2, :],
                        psv[:N_TOK, :nb2 * DIM].rearrange(
                            "p (j c) -> p j c", c=DIM
                        ),
                    )
                return v_sb

            def emit_scores(qk_sb, g):
                e_tiles = []   # (e_tile, fill_start_batch_local)
                for f0 in range(0, g, FILL):
                    nb = min(FILL, g - f0)
                    s_ps = s_ps_pool.tile([N_TOK, 2048], F32)
                    for jl in range(nb):
                        j = f0 + jl
                        for h in range(H):
                            bank = h % 4
                            slot = 2 * jl + h // 4
                            hq = h // 4
                            hp = 32 * (h % 4)
                            nc.tensor.matmul(
                                s_ps[:, 512 * bank + 84 * slot:
                                     512 * bank + 84 * slot + 84],
                                qk_sb[hp:hp + 32, 2 + hq,
                                      j * N_TOK:(j + 1) * N_TOK],
                                qk_sb[hp:hp + 32, 0 + hq,
                                      j * N_TOK:(j + 1) * N_TOK],
                                start=True, stop=True,
                                tile_position=(hp, 0),
                            )
                    vcols = 84 * 2 * nb
                    s_view = s_ps.rearrange("p (bk c) -> p bk c", c=512)
                    e0 = e0_pool.tile([N_TOK, 4, 504], BF16, tag="e0")
                    nc.scalar.activation(
                        e0[:, :, :vcols], s_view[:, :, :vcols],
                        mybir.ActivationFunctionType.Exp, scale=SCALE,
                    )
                    e = e_pool.tile([N_TOK, 4, 504], BF16, tag="e")
                    nc.vector.tensor_tensor(
                        e[:, :, :vcols], e0[:, :, :vcols], eb_sb[:, :, :vcols],
                        mybir.AluOpType.mult,
                    )
                    e_tiles.append((e, f0))
                return e_tiles

            def emit_av(prev, hg):
                _, g, v_sb, e_tiles = prev
                TG = g * N_TOK
                av = av_ps_pool.tile([128, 1024], F32)
                for j in range(g):
                    e, f0 = e_tiles[j // FILL]
                    jl = j - f0
                    for hh in range(4):
                        h = 4 * hg + hh
                        bank = h % 4
                        slot = 2 * jl + h // 4
                        esl = e[:, bank, 84 * slot:84 * slot + 84]
                        # O_un^T block
                        nc.tensor.matmul(
                            av[32 * hh:32 * hh + 32, 84 * j:84 * j + 84],
                            v_sb[:, j, 32 * h:32 * h + 32],
                            esl, start=True, stop=True,
                            tile_position=(0, 32 * hh),
                        )
                        # denominator replicated over the 32 partitions
                        nc.tensor.matmul(
                            av[32 * hh:32 * hh + 32,
                               512 + 84 * j:512 + 84 * j + 84],
                            ones_sb, esl, start=True, stop=True,
                            tile_position=(0, 32 * hh),
                        )
                r_sb = ot_pool.tile([128, GROUP * N_TOK], F32, tag=f"d{hg}")
                nc.vector.reciprocal_approx_fast(
                    r_sb[:, :TG], av[:, 512:512 + TG])
                ot = ot_pool.tile([128, GROUP * N_TOK], BF16, tag=f"ot{hg}")
                nc.vector.tensor_tensor(
                    ot[:, :TG], av[:, :TG], r_sb[:, :TG],
                    mybir.AluOpType.mult,
                )
                return ot

            def emit_proj(prev, ot_tiles):
                g0, g, _, _ = prev
                TG = g * N_TOK
                T0 = g0 * N_TOK
                y_sb = y_pool.tile([128, 2, GROUP * N_TOK], F32)
                for m in range(2):
                    psy = mm_ps.tile([128, 512], F32, tag="mmps")
                    for kc in range(2):
                        nc.tensor.matmul(
                            psy[:, :TG],
                            wproj_sb[:, kc, m * 128:(m + 1) * 128],
                            ot_tiles[kc][:, :TG],
                            start=(kc == 0), stop=(kc == 1),
                        )
                    nc.any.tensor_copy(y_sb[:, m, :TG], psy[:, :TG])
                nc.sync.dma_start(
                    yT[:, T0:T0 + TG].rearrange("(m p) t -> p m t", p=128),
                    y_sb[:, :, :TG],
                )

            # ---------- software-pipelined main loop (1-group skew) ----------
            # PE program order interleaves group i's qkv/v/QK matmuls with
            # group i-1's AV/proj so the PE never sits behind ACT/DVE.
            prev = None
            for (g0, g) in groups:
                xt, qk_sb = emit_qkv(g0, g)
                ot1 = emit_av(prev, 0) if prev else None
                v_sb = emit_v(xt, g)
                ot2 = emit_av(prev, 1) if prev else None
                e_tiles = emit_scores(qk_sb, g)
                if prev:
                    emit_proj(prev, [ot1, ot2])
                prev = (g0, g, v_sb, e_tiles)
            ot1 = emit_av(prev, 0)
            ot2 = emit_av(prev, 1)
            emit_proj(prev, [ot1, ot2])

    nc.compile()
    return nc


def _host_prep(x, W_qkv, b_qkv, W_proj, b_proj, bias_table, rel_index, n_batches):
    """Build the per-core input dict pieces shared across cores."""
    bias_full = bias_table[rel_index]          # (84, 84, 8): [q, k, h]
    bias_full = np.transpose(bias_full, (2, 0, 1))  # [h, q, k]
    eb = np.zeros((N_TOK, 4, 504), dtype=np.float32)
    for bank in range(4):
        for slot in range(6):
            h = bank + 4 * (slot % 2)
            # eb[k, bank, 84*slot + q] = exp(bias[h, q, k])
            eb[:, bank, 84 * slot:84 * slot + 84] = np.exp(bias_full[h].T)
    eb = eb.astype(ml_dtypes.bfloat16)

    shared = {
        "wqkv": np.ascontiguousarray(W_qkv).astype(ml_dtypes.bfloat16),
        "bqkv": np.ascontiguousarray(b_qkv, dtype=np.float32),
        "wproj": np.ascontiguousarray(W_proj).astype(ml_dtypes.bfloat16),
        "ebias": eb,
    }
    return shared


_NC_CACHE = {}


def kernel(x, W_qkv, b_qkv, W_proj, b_proj, bias_table, rel_index,
           n_batches_per_core=B_SHARD):
    x = np.asarray(x, dtype=np.float32)
    W_qkv = np.asarray(W_qkv, dtype=np.float32)
    b_qkv = np.asarray(b_qkv, dtype=np.float32)
    W_proj = np.asarray(W_proj, dtype=np.float32)
    b_proj = np.asarray(b_proj, dtype=np.float32)
    bias_table = np.asarray(bias_table, dtype=np.float32)
    rel_index = np.asarray(rel_index)

    nb = n_batches_per_core
    shared = _host_prep(x, W_qkv, b_qkv, W_proj, b_proj, bias_table,
                        rel_index, nb)

    if nb not in _NC_CACHE:
        _NC_CACHE[nb] = build_nc(nb)
    nc = _NC_CACHE[nb]

    in_maps = []
    for c in range(N_CORES):
        xs = x[c * nb:(c + 1) * nb].reshape(nb * N_TOK, DIM)
        xTc = np.ascontiguousarray(xs.T).astype(ml_dtypes.bfloat16)
        m = dict(shared)
        m["xT"] = xTc
        in_maps.append(m)

    res = run_bass_kernel_spmd(nc, in_maps, core_ids=list(range(N_CORES)))

    # combined output bias: b_proj plus v-bias passed through attention
    bias_out = (b_qkv[2 * DIM:3 * DIM] @ W_proj + b_proj).astype(np.float32)

    outs = []
    for c in range(N_CORES):
        yTc = res.results[c]["yT"]               # (256, nb*84)
        y = np.ascontiguousarray(yTc.T).reshape(nb, N_TOK, DIM)
        outs.append(y)
    out = np.concatenate(outs, axis=0)
    out += bias_out[None, None, :]
    return out.astype(np.float32)


if __name__ == "__main__":
    # smoke test with tiny batch count per core
    rng = np.random.default_rng(0)
    nb = 6
    B = N_CORES * nb
    x = rng.standard_normal((B, N_TOK, DIM), dtype=np.float32)
    W_qkv = rng.standard_normal((DIM, 3 * DIM), dtype=np.float32) * 0.02
    b_qkv = np.zeros(3 * DIM, np.float32)
    W_proj = rng.standard_normal((DIM, DIM), dtype=np.float32) * 0.02
    b_proj = np.zeros(DIM, np.float32)
    bias_table = rng.standard_normal((207, H), dtype=np.float32) * 0.02
    # rebuild rel_index like the reference
    SQ = [64, 16, 4]
    offset = [0]
    for i in range(2):
        offset.append(sum(SQ[-i - 1:]))
    off_h = np.concatenate([np.full(SQ[i], offset[i], dtype=np.int64) for i in range(3)])
    off_w = np.concatenate([np.full(SQ[i], offset[-i - 1], dtype=np.int64) for i in range(3)])
    ch = np.arange(N_TOK)
    cw = np.arange(N_TOK)[::-1]
    rel_index = (ch[:, None] + cw[None, :] + off_h[:, None] + off_w[None, :]).astype(np.int32)

    out = kernel(x, W_qkv, b_qkv, W_proj, b_proj, bias_table, rel_index,
                 n_batches_per_core=nb)
    print("out", out.shape, out.dtype)
